# revision 15
# baseline (speedup 1.0000x reference)
"""LSTM encoder (T=512, B=256, H=256, V=32000) on 8 trn2 NeuronCores.

Strategy
--------
Data-parallel over batch: B=256 -> 32 per core; weights/table replicated.

Per core the recurrence runs in a transposed "gatesT" layout: gates live as
[4H on partitions (8 chunks of 128), batch in the free dim]. Weight chunks
are the stationary matmul operand (fp16), h streams as the moving operand.

The per-core batch of 32 is split into TWO independent 16-lane chains that
run interleaved. Wall time = one chain's serial step latency, so the
schedule minimizes that loop:

  burst(16 mm) -> sigmoid -> Y=(sg-0.5)*si -> f*C -> add -> tanh -> o*tc

* State is stored halved (C = c/2) so one scalar_tensor_tensor computes
  i*tanh(g)/2 from raw sigmoid outputs (tanh(x) = 2 sig(2x) - 1, g rows
  pre-scaled by 2 host-side), and the final tanh(c) = Tanh(C, scale=2).
* All elementwise state is fp16 (DVE 2x/4x modes, fp16 error decays
  geometrically through the forget gate).
* PE p-state: the recurrent bursts only run fast if the PE never idles.
  The W_ih @ emb window precompute and the bias fills are granulated into
  N=64/N=128 matmuls and spread across every step of the previous window,
  topped up with zero-weight dummy matmuls, so the PE stays warm into
  each latency-critical burst.

Embeddings are fetched with dma_gather(transpose=True): gathers fp16 table
rows and deposits them H-on-partitions, the exact rhs layout the
X-projection matmuls need. Gate chunk order is [f, i, o, g].

Numerics: fp16 table/weights/h/c. Measured rel err ~1e-3 vs fp32 ref.
"""

import numpy as np

T, B, H, V = 512, 256, 256, 32000
N_CORES = 8
BL = B // N_CORES          # 32 batch per core
HB = BL // 2               # 16 per half-chain
S = 8                      # steps per PSUM window
G4 = 4 * H                 # 1024
M = G4 // 128              # 8 gate chunks
K = H // 128               # 2 contraction chunks
DUMN = 7                   # dummy warm-up matmuls per step

# gate chunk order f, i, o, g (PyTorch native is i, f, g, o)
_PERM = np.concatenate([
    np.arange(H, 2 * H),       # f
    np.arange(0, H),           # i
    np.arange(3 * H, 4 * H),   # o
    np.arange(2 * H, 3 * H),   # g
])


def _build_bass(t_steps=T):
    from contextlib import ExitStack
    from concourse import bacc, mybir, library_config
    import concourse.tile as tile

    f16, f32, i16 = mybir.dt.float16, mybir.dt.float32, mybir.dt.int16
    Sig = mybir.ActivationFunctionType.Sigmoid
    Tanh = mybir.ActivationFunctionType.Tanh
    mult, add = mybir.AluOpType.mult, mybir.AluOpType.add
    sub = mybir.AluOpType.subtract

    NW = t_steps // S
    NI = S * BL            # 256 gathered rows per window (both halves)

    nc = bacc.Bacc("TRN2", target_bir_lowering=False, debug=False)
    idx_d = nc.declare_dram_parameter("idx", [128, NW, NI // 16], i16, isOutput=False)
    tab_d = nc.declare_dram_parameter("table", [V, H], f16, isOutput=False)
    wih_d = nc.declare_dram_parameter("wih_t", [H, G4], f16, isOutput=False)
    whh_d = nc.declare_dram_parameter("whh_t", [H, G4], f16, isOutput=False)
    b_d = nc.declare_dram_parameter("bias", [1, G4], f16, isOutput=False)
    on_d = nc.declare_dram_parameter("ones", [1, S * HB], f16, isOutput=False)
    z_d = nc.declare_dram_parameter("zeros", [128, 128], f16, isOutput=False)
    h0_d = nc.declare_dram_parameter("h0t", [128, K, BL], f16, isOutput=False)
    c0_d = nc.declare_dram_parameter("c0t", [128, K, BL], f16, isOutput=False)
    ho_d = nc.declare_dram_parameter("h_out", [128, K, BL], f32, isOutput=True)
    co_d = nc.declare_dram_parameter("c_out", [128, K, BL], f16, isOutput=True)

    import bass_rust

    def pin(ins_i, after_i, why="pin", sync=False):
        if after_i is not None:
            bass_rust.add_dep_helper(ins_i.ins, after_i.ins, sync=sync,
                                     reason=why)

    with tile.TileContext(nc) as tc, ExitStack() as ctx:
        const = ctx.enter_context(tc.tile_pool(name="const", bufs=1))
        embp = ctx.enter_context(tc.tile_pool(name="embp", bufs=3))
        psum = ctx.enter_context(tc.tile_pool(name="psum", bufs=2, space="PSUM"))
        sp = ctx.enter_context(tc.tile_pool(name="sp", bufs=3))
        tmp = ctx.enter_context(tc.tile_pool(name="tmp", bufs=3))
        hp = ctx.enter_context(tc.tile_pool(name="hp", bufs=3))

        # idx upload + library load first so the first gather's Q7 work
        # overlaps the remaining constant DMAs
        idx_sb = const.tile([128, NW, NI // 16], i16, name="idx_sb")
        nc.sync.dma_start(idx_sb[:], idx_d[:])
        nc.gpsimd.load_library(library_config.mlp)
        whh_sb, wih_sb = [], []
        for k in range(K):
            wt = const.tile([128, G4], f16, name=f"whh_sb{k}")
            nc.sync.dma_start(wt[:], whh_d[128 * k:128 * (k + 1), :])
            whh_sb.append(wt)
            xt = const.tile([128, G4], f16, name=f"wih_sb{k}")
            nc.sync.dma_start(xt[:], wih_d[128 * k:128 * (k + 1), :])
            wih_sb.append(xt)
        b_sb = const.tile([1, G4], f16, name="b_sb")
        nc.sync.dma_start(b_sb[:], b_d[:])
        on_sb = const.tile([1, S * HB], f16, name="on_sb")
        nc.sync.dma_start(on_sb[:], on_d[:])
        z_sb = const.tile([128, 128], f16, name="z_sb")
        nc.sync.dma_start(z_sb[:], z_d[:])

        # per-half state: C = c/2 (fp16), h (fp16)
        ct, h_cur = [], []
        for ha in range(2):
            c_t = const.tile([128, K, HB], f16, name=f"ct{ha}")
            nc.sync.dma_start(c_t[:], c0_d[:, :, HB * ha:HB * (ha + 1)])
            ct.append(c_t)
            h0 = const.tile([128, K, HB], f16, name=f"h0_sb{ha}")
            nc.sync.dma_start(h0[:], h0_d[:, :, HB * ha:HB * (ha + 1)])
            h_cur.append(h0)

        embt = {}
        ps = {}

        def gather(w):
            # one gather per window; half A = cols 0:128, half B = 128:256
            e = embp.tile([128, K, NI], f16, name="embt", tag=f"embt{w % 3}",
                          bufs=1)
            g_i = nc.gpsimd.dma_gather(
                out_ap=e[:], in_ap=tab_d[:],
                idxs_ap=idx_sb[:, w, :],
                num_idxs=NI, num_idxs_reg=NI, elem_size=H, transpose=True)
            embt[w] = e
            return g_i

        def alloc_ps(w, ha):
            p = psum.tile([128, M, S, HB], f32, name="ps",
                          tag=f"ps{ha}_{w % 2}", bufs=1)
            ps[(w, ha)] = p

        def filler_ops(w):
            """All PE fill work for window w's PSUM: bias granules (N=128,
            start=True per chunk) then X granules (N=64). Returned as a list
            of closures so the caller can spread them across steps."""
            ops = []
            for ha in range(2):
                for m in range(M):
                    # start=True marks the whole 2KB bank pending-zero, so
                    # only the first chunk of each bank starts; later writes
                    # into the pending region overwrite (not accumulate).
                    for q in range(2):
                        def bias_g(ha=ha, m=m, q=q):
                            return nc.tensor.matmul(
                                out=ps[(w, ha)][:, m, 4 * q:4 * (q + 1), :],
                                lhsT=b_sb[0:1, 128 * m:128 * (m + 1)],
                                rhs=on_sb[0:1, 64 * q:64 * (q + 1)],
                                start=(m % 4 == 0 and q == 0), stop=False,
                                skip_group_check=True)
                        ops.append(bias_g)
            for k in range(K):
                for m in range(M):
                    for ha in range(2):
                        for q in range(4):
                            def x_g(ha=ha, m=m, k=k, q=q):
                                return nc.tensor.matmul(
                                    out=ps[(w, ha)][:, m, 2 * q:2 * (q + 1), :],
                                    lhsT=wih_sb[k][:, 128 * m:128 * (m + 1)],
                                    rhs=embt[w][:, k,
                                                128 * ha + 32 * q:
                                                128 * ha + 32 * (q + 1)],
                                    start=False, stop=False,
                                    skip_group_check=True)
                            ops.append(x_g)
            return ops

        def dummy_op(w, ha, m):
            # zero-weight matmul: numeric no-op PE warmth into a live bank
            return nc.tensor.matmul(
                out=ps[(w, ha)][:, m, 0:4, :],
                lhsT=z_sb[:, :],
                rhs=embt[w][:, 0, 0:64],
                start=False, stop=False, skip_group_check=True)

        def burst(w, s, ha):
            last = None
            for k in range(K):
                for m in range(M):
                    last = nc.tensor.matmul(
                        out=ps[(w, ha)][:, m, s, :],
                        lhsT=whh_sb[k][:, 128 * m:128 * (m + 1)],
                        rhs=h_cur[ha][:, k, :],
                        start=False, stop=(k == K - 1), skip_group_check=True)
            return last

        # prologue: window 0 fully prepared, window 1 gathered
        gather(0)
        if NW > 1:
            gather(1)
        for ha in range(2):
            alloc_ps(0, ha)
        for op in filler_ops(0):
            op()

        pend = []              # filler closures for window w+1

        for w in range(NW):
            if w + 1 < NW:
                alloc_ps(w + 1, 0)
                alloc_ps(w + 1, 1)
                pend = filler_ops(w + 1)
            else:
                pend = []
            for s in range(S):
                t = w * S + s
                burst(w, s, 0)
                last_mm = burst(w, s, 1)
                sall = []
                for ha in range(2):
                    sa = sp.tile([128, M, HB], f16, name="sall", tag=f"sall{ha}")
                    sall.append(sa)
                nc.scalar.activation(sall[0][:], ps[(w, 0)][:, :, s, :], Sig)
                nc.scalar.activation(sall[1][:], ps[(w, 1)][:, :, s, :], Sig)

                yt = [tmp.tile([128, K, HB], f16, name="yt", tag=f"yt{ha}")
                      for ha in range(2)]
                fct = [tmp.tile([128, K, HB], f16, name="fct", tag=f"fct{ha}")
                       for ha in range(2)]
                tch = [tmp.tile([128, K, HB], f16, name="tct", tag=f"tct{ha}")
                       for ha in range(2)]

                def cell(ha):
                    # Y = (sig(2g) - 0.5) * sig(i) = i*tanh(g)/2
                    nc.vector.scalar_tensor_tensor(
                        yt[ha][:], sall[ha][:, 6:8, :], 0.5,
                        sall[ha][:, 2:4, :], sub, mult)
                    # f * C
                    nc.vector.tensor_tensor(
                        fct[ha][:], sall[ha][:, 0:2, :], ct[ha][:], mult)

                addi = [None, None]

                def cell2(ha):
                    # C_new = f*C + Y  (= c_new / 2)
                    addi[ha] = nc.vector.tensor_tensor(
                        ct[ha][:], fct[ha][:], yt[ha][:], add)
                    # tanh(c) = Tanh(2*C)
                    nc.scalar.activation(tch[ha][:], ct[ha][:], Tanh, scale=2.0)

                def h_update(ha):
                    # k-split h = o * tanh(c) so the next burst's k=0
                    # matmuls start one DVE op earlier
                    if t < t_steps - 1:
                        hn = hp.tile([128, K, HB], f16, name="hn", tag=f"hn{ha}")
                        for k in range(K):
                            nc.vector.tensor_tensor(
                                hn[:, k, :], sall[ha][:, 4 + k, :],
                                tch[ha][:, k, :], mult)
                        h_cur[ha] = hn
                        return
                    hf = tmp.tile([128, K, HB], f32, name="hf", tag=f"hf{ha}")
                    nc.vector.tensor_tensor(
                        hf[:], sall[ha][:, 4:6, :], tch[ha][:], mult)
                    nc.sync.dma_start(ho_d[:, :, HB * ha:HB * (ha + 1)], hf[:])
                    nc.sync.dma_start(co_d[:, :, HB * ha:HB * (ha + 1)],
                                      ct[ha][:])

                # DVE queue order: A's full tail first; A's h-mul slots in
                # right after B's Y/fc (which are ready earlier) so it
                # doesn't stall behind B's add, and B's add/h follow.
                cell(0)
                cell2(0)
                cell(1)
                h_update(0)
                cell2(1)
                h_update(1)

                # PE warmth: spread window w+1's bias/X granules across this
                # window's steps, topped with zero-weight dummies, all pinned
                # behind this step's recurrent burst so they fill (and heat)
                # the PE-idle tail without delaying the burst.
                if w + 1 < NW and s == 0:
                    g_i = gather(w + 2) if w + 2 < NW else None
                n_f = len(pend)
                lo = n_f * s // S
                hi = n_f * (s + 1) // S
                prev = last_mm
                for j in range(lo, hi):
                    mm = pend[j]()
                    pin(mm, prev, "fill after burst")
                    if j == lo:
                        # hold the whole filler slice until this step's
                        # c-update: it then runs compactly in the pre-burst
                        # window, re-heating the PE right before the next
                        # latency-critical burst instead of finishing early
                        # and letting it cool.
                        pin(mm, addi[0], "delay filler to pre-burst",
                            sync=True)
                    prev = mm
            if w > 0:
                for ha in range(2):
                    ps.pop((w - 1, ha), None)
                embt.pop(w - 1, None)
    nc.finalize()
    return nc


def _prep_inputs(enc_inputs, h0, c0, embed, W_ih, W_hh, b_ih, b_hh, t_steps=T):
    """Host-side shard + layout prep. Returns list of per-core in_maps."""
    Wih_p = W_ih[_PERM].astype(np.float32).copy()
    Whh_p = W_hh[_PERM].astype(np.float32).copy()
    b_p = (b_ih + b_hh)[_PERM].astype(np.float32).copy()
    # g rows pre-scaled by 2: tanh(x) = 2*sigmoid(2x) - 1
    Wih_p[3 * H:] *= 2.0
    Whh_p[3 * H:] *= 2.0
    b_p[3 * H:] *= 2.0
    wih_t = np.ascontiguousarray(Wih_p.T).astype(np.float16)   # [H, 4H]
    whh_t = np.ascontiguousarray(Whh_p.T).astype(np.float16)
    bias = b_p.astype(np.float16).reshape(1, G4)
    table = embed.astype(np.float16)                           # [V, H]
    ones = np.ones((1, S * HB), np.float16)
    zeros = np.zeros((128, 128), np.float16)

    NW = t_steps // S
    in_maps = []
    for c in range(N_CORES):
        wrapped = np.empty((128, NW, S * BL // 16), np.int16)
        for w in range(NW):
            # window's 256 indices: half A block then half B block, t-major
            blocks = []
            for ha in range(2):
                bs = slice(c * BL + HB * ha, c * BL + HB * (ha + 1))
                blocks.append(
                    enc_inputs[w * S:(w + 1) * S, bs].astype(np.int16).reshape(-1))
            flat = np.concatenate(blocks)                      # [256]
            w16 = flat.reshape(-1, 16).T                       # [16, 16]
            wrapped[:, w, :] = np.tile(w16, (8, 1))
        bs = slice(c * BL, (c + 1) * BL)
        h0t = np.empty((128, K, BL), np.float16)
        c0t = np.empty((128, K, BL), np.float16)
        for k in range(K):
            h0t[:, k, :] = h0[bs].T[128 * k:128 * (k + 1), :]
            c0t[:, k, :] = (0.5 * c0[bs]).T[128 * k:128 * (k + 1), :]
        in_maps.append({
            "idx": np.ascontiguousarray(wrapped), "table": table,
            "wih_t": wih_t, "whh_t": whh_t,
            "bias": bias, "ones": ones, "zeros": zeros,
            "h0t": h0t, "c0t": c0t,
        })
    return in_maps


def _unshard(results):
    h = np.empty((B, H), np.float32)
    c = np.empty((B, H), np.float32)
    for core, out in enumerate(results):
        bs = slice(core * BL, (core + 1) * BL)
        for k in range(K):
            h[bs, 128 * k:128 * (k + 1)] = out["h_out"][:, k, :].T
            c[bs, 128 * k:128 * (k + 1)] = (
                2.0 * out["c_out"][:, k, :].astype(np.float32)).T
    return h, c


def kernel(enc_inputs, h0, c0, embed, W_ih, W_hh, b_ih, b_hh):
    from concourse.bass_utils import run_bass_kernel_spmd

    enc_inputs = np.asarray(enc_inputs)
    h0 = np.asarray(h0, dtype=np.float32)
    c0 = np.asarray(c0, dtype=np.float32)
    embed = np.asarray(embed, dtype=np.float32)
    W_ih = np.asarray(W_ih, dtype=np.float32)
    W_hh = np.asarray(W_hh, dtype=np.float32)
    b_ih = np.asarray(b_ih, dtype=np.float32)
    b_hh = np.asarray(b_hh, dtype=np.float32)

    nc = _build_bass()
    in_maps = _prep_inputs(enc_inputs, h0, c0, embed, W_ih, W_hh, b_ih, b_hh)
    res = run_bass_kernel_spmd(nc, in_maps, core_ids=list(range(N_CORES)))
    return _unshard(res.results)
